# revision 1
# baseline (speedup 1.0000x reference)
"""Gumbel-softmax VQ codebook kernel for 8 TRN2 NeuronCores.

Math (per group g of 4, sub_D=128, K=512, rows n = flattened B*T):
    dis[g,n,k]  = ||W_g[k]||^2 + ||x_g[n]||^2 - 2 x_g[n].W_g[k]
    enc         = softmax_k((-dis + gum)/tau),  tau = 2
    quant[g,n]  = enc[g,n,:] @ W_g
    avg_probs   = mean_n enc ;  perp = sum_g exp(-sum_k avg log(avg+1e-10))

Key simplifications used here:
  * softmax is shift-invariant, so the ||x||^2 term (constant in k) drops.
  * logits/tau = x.W - ||W||^2/2 + gum/2 which is <= ~8.5, so exp() is safe
    in fp32 without the max-subtraction pass.
  * x arrives as [B, D, T]: for each (batch, group) the slice x[b, g*128:(g+1)*128, :]
    IS x_g^T (d on partitions) -- no input transpose needed anywhere.
  * enc^T (needed for the quant matmul) is produced by a PE matmul against
    diag(1/s), fusing the softmax normalization into the transpose.
  * the per-k column sums of enc (for avg_probs) ride the enc^T PSUM->SBUF
    copies via the scalar engine's accum_out, then 8 partial sums are
    combined on the host (tiny [4,512] reduction).

Sharding: data-parallel over rows; core c takes batches {2c, 2c+1}
(= flat rows [c*4096, (c+1)*4096)); the four [512,128] codebooks are
replicated.
"""

import threading

import numpy as np

import concourse.bacc as bacc
from concourse import mybir
from concourse.bass_utils import run_bass_kernel_spmd
from concourse.masks import make_identity
from concourse.tile import TileContext

FP32 = mybir.dt.float32
BF16 = mybir.dt.bfloat16

B, D, T = 16, 512, 2048
N_D, K, SUB_D = 4, 512, 128
NCORES = 8
BPC = B // NCORES          # batches per core = 2
ROWS = BPC * T             # rows per core = 4096
N_TOTAL = B * T            # 32768


def _build_nc():
    nc = bacc.Bacc("TRN2", target_bir_lowering=False, debug=False,
                   num_devices=NCORES)
    x = nc.dram_tensor("x", [BPC, D, T], FP32, kind="ExternalInput")
    gum = nc.dram_tensor("gum", [N_D, ROWS, K], FP32, kind="ExternalInput")
    w = nc.dram_tensor("w", [N_D, K, SUB_D], FP32, kind="ExternalInput")
    quant = nc.dram_tensor("quant", [BPC, D, T], FP32, kind="ExternalOutput")
    # sums[g, p, blk] = sum_n enc[g, n, blk*128 + p]
    sums = nc.dram_tensor("sums", [N_D, 128, 4], FP32, kind="ExternalOutput")

    xa, ga, wa, qa, sa = x.ap(), gum.ap(), w.ap(), quant.ap(), sums.ap()

    with TileContext(nc) as tc:
        with (
            tc.tile_pool(name="const", bufs=1) as constp,
            tc.tile_pool(name="wg", bufs=2) as wgp,
            tc.tile_pool(name="xin", bufs=2) as xp,
            tc.tile_pool(name="gin", bufs=4) as gp,
            tc.tile_pool(name="work", bufs=4) as wp,
            tc.tile_pool(name="qo", bufs=3) as qp,
            tc.tile_pool(name="small", bufs=6) as sp,
            tc.tile_pool(name="ps", bufs=2, space="PSUM") as psp,
        ):
            ident = constp.tile([128, 128], BF16)
            make_identity(nc, ident)
            ones_f32 = constp.tile([128, 1], FP32)
            nc.vector.memset(ones_f32, 1.0)
            ones_bf = constp.tile([1, 128], BF16)
            nc.vector.memset(ones_bf, 1.0)

            for g in range(N_D):
                # ---- per-group setup: codebook tiles + column norms ----
                w_f32 = wgp.tile([128, 4, 128], FP32, tag="w_f32")
                nc.sync.dma_start(
                    out=w_f32,
                    in_=wa[g].rearrange("(blk p) d -> p blk d", p=128),
                )
                # w_bf[p, blk, d] = W[blk*128+p, d]  (quant lhsT slices)
                w_bf = wgp.tile([128, 4, 128], BF16, tag="w_bf")
                nc.vector.tensor_copy(w_bf, w_f32)
                # wgT[d, blk, p] = W[blk*128+p, d]; view [128, 512] = W^T
                wgT = wgp.tile([128, 4, 128], BF16, tag="wgT")
                for blk in range(4):
                    tps = psp.tile([128, 128], BF16, tag="tps", bufs=1)
                    nc.tensor.transpose(tps, w_bf[:, blk, :], ident)
                    nc.scalar.copy(wgT[:, blk, :], tps)
                wgT_v = wgT.rearrange("p blk d -> p (blk d)")
                # cs_neg_half[1, k] = -||W_k||^2 / 2 via ones^T @ (W^T)^2
                sq = wgp.tile([128, 4, 128], FP32, tag="sq")
                nc.vector.tensor_mul(sq, wgT, wgT)
                cs_ps = psp.tile([1, 512], FP32, tag="cs_ps", bufs=1)
                nc.tensor.matmul(cs_ps, lhsT=ones_f32,
                                 rhs=sq.rearrange("p blk d -> p (blk d)"),
                                 start=True, stop=True)
                csnh = wgp.tile([1, 512], BF16, tag="csnh")
                nc.scalar.mul(csnh, cs_ps, -0.5)
                # running per-k sums of enc for this group
                sums_sb = wgp.tile([128, 4], FP32, tag="sums_sb")
                nc.vector.memset(sums_sb, 0.0)

                for b in range(BPC):
                    xg_f32 = xp.tile([128, T], FP32, tag="xg_f32")
                    nc.sync.dma_start(out=xg_f32,
                                      in_=xa[b, g * 128:(g + 1) * 128, :])
                    xg_bf = xp.tile([128, T], BF16, tag="xg_bf")
                    nc.vector.tensor_copy(xg_bf, xg_f32)
                    for tch in range(T // 512):
                        qout = qp.tile([128, 512], FP32, tag="qout")
                        for i in range(4):
                            t0 = tch * 512 + i * 128
                            n0 = b * T + t0
                            gum_t = gp.tile([128, 512], FP32, tag="gum_t")
                            nc.sync.dma_start(out=gum_t,
                                              in_=ga[g, n0:n0 + 128, :])
                            # logits*tau' in PSUM: x.W then rank-1 -||W||^2/2
                            lps = psp.tile([128, 512], FP32, tag="lps")
                            nc.tensor.matmul(lps, lhsT=xg_bf[:, t0:t0 + 128],
                                             rhs=wgT_v, start=True, stop=False)
                            nc.tensor.matmul(lps, lhsT=ones_bf, rhs=csnh,
                                             start=False, stop=True)
                            # u = gum/2 + (x.W - cs/2) ; e = exp(u), s = row sums
                            u_sb = wp.tile([128, 512], FP32, tag="u_sb")
                            nc.vector.scalar_tensor_tensor(
                                out=u_sb, in0=gum_t, scalar=0.5, in1=lps,
                                op0=mybir.AluOpType.mult,
                                op1=mybir.AluOpType.add)
                            e_bf = wp.tile([128, 512], BF16, tag="e_bf")
                            s_f32 = sp.tile([128, 1], FP32, tag="s_f32")
                            nc.scalar.activation(
                                e_bf, u_sb, mybir.ActivationFunctionType.Exp,
                                accum_out=s_f32)
                            r_f32 = sp.tile([128, 1], FP32, tag="r_f32")
                            nc.vector.reciprocal(r_f32, s_f32)
                            diag = sp.tile([128, 128], BF16, tag="diag")
                            nc.vector.tensor_scalar_mul(diag, ident, r_f32)
                            # normalized transpose: encT = e^T @ diag(r)
                            eT_ps = psp.tile([128, 512], FP32, tag="eT_ps")
                            eT_sb = wp.tile([128, 4, 128], BF16, tag="eT_sb")
                            asum = sp.tile([128, 4], FP32, tag="asum")
                            qps = psp.tile([128, 128], FP32, tag="qps")
                            for blk in range(4):
                                sl = slice(blk * 128, (blk + 1) * 128)
                                nc.tensor.matmul(eT_ps[:, sl],
                                                 lhsT=e_bf[:, sl], rhs=diag,
                                                 start=True, stop=True)
                                nc.scalar.activation(
                                    eT_sb[:, blk, :], eT_ps[:, sl],
                                    mybir.ActivationFunctionType.Copy,
                                    accum_out=asum[:, blk:blk + 1])
                                nc.tensor.matmul(qps, lhsT=w_bf[:, blk, :],
                                                 rhs=eT_sb[:, blk, :],
                                                 start=(blk == 0),
                                                 stop=(blk == 3))
                            nc.vector.tensor_add(sums_sb, sums_sb, asum)
                            nc.scalar.copy(qout[:, i * 128:(i + 1) * 128], qps)
                        nc.sync.dma_start(
                            out=qa[b, g * 128:(g + 1) * 128,
                                   tch * 512:(tch + 1) * 512],
                            in_=qout)
                nc.sync.dma_start(out=sa[g], in_=sums_sb)

    nc.compile()
    return nc


_nc_lock = threading.Lock()
_nc_cache = []


def _get_nc():
    with _nc_lock:
        if not _nc_cache:
            _nc_cache.append(_build_nc())
        return _nc_cache[0]


def kernel(x, W1, W2, W3, W4, gumbels, _trace=False):
    nc = _get_nc()
    W = np.ascontiguousarray(np.stack([W1, W2, W3, W4], 0), dtype=np.float32)
    in_maps = []
    for c in range(NCORES):
        in_maps.append({
            "x": np.ascontiguousarray(x[BPC * c:BPC * (c + 1)]),
            "gum": np.ascontiguousarray(gumbels[:, ROWS * c:ROWS * (c + 1), :]),
            "w": W,
        })
    res = run_bass_kernel_spmd(nc, in_maps, core_ids=list(range(NCORES)),
                               trace=_trace)
    quant = np.empty((B, D, T), np.float32)
    sums_total = np.zeros((N_D, 128, 4), np.float64)
    for c, r in enumerate(res.results):
        quant[BPC * c:BPC * (c + 1)] = r["quant"]
        sums_total += r["sums"]
    avg = sums_total.transpose(0, 2, 1).reshape(N_D, K) / float(N_TOTAL)
    perp = np.sum(np.exp(-np.sum(avg * np.log(avg + 1e-10), axis=-1)))
    out = quant, np.float32(perp)
    if _trace:
        return out, res
    return out


# revision 2
# speedup vs baseline: 1.0970x; 1.0970x over previous
"""Gumbel-softmax VQ codebook kernel for 8 TRN2 NeuronCores.

Math (per group g of 4, sub_D=128, K=512, rows n = flattened B*T):
    dis[g,n,k]  = ||W_g[k]||^2 + ||x_g[n]||^2 - 2 x_g[n].W_g[k]
    enc         = softmax_k((-dis + gum)/tau),  tau = 2
    quant[g,n]  = enc[g,n,:] @ W_g
    avg_probs   = mean_n enc ;  perp = sum_g exp(-sum_k avg log(avg+1e-10))

Design notes:
  * softmax is shift-invariant, so the ||x||^2 term (constant in k) drops;
    logits/tau = x.W - ||W||^2/2 + gum/2 <= ~8.5, so exp() is safe in fp32
    without a max-subtraction pass.
  * x arrives [B, D, T]: x[b, g*128:(g+1)*128, :] IS x_g^T (d on partitions)
    -- no input transposes anywhere.
  * enc^T (for the quant matmul) comes from a PE matmul against diag(1/s),
    fusing softmax normalization into the transpose.
  * avg_probs rides a rank-1 PE matmul (lhsT = 1/s) accumulating into one
    PSUM bank per group; host combines the 8 per-core partial sums.
  * per-instruction overheads dominate small ops, so: the e^T PSUM->SBUF
    copy is ONE [128,512] op alternating DVE/ACT, quant copies are batched
    over 4 row-tiles, the gum+logits add is pair-batched on DVE, and tiny
    ops (diag build, casts) go to the idle GpSimd engine.

Sharding: data-parallel over rows; core c takes batches {2c, 2c+1}
(= flat rows [c*4096, (c+1)*4096)); codebooks replicated.
"""

import threading

import numpy as np

import concourse.bacc as bacc
from concourse import mybir
from concourse.bass_utils import run_bass_kernel_spmd
from concourse.masks import make_identity
from concourse.tile import TileContext

FP32 = mybir.dt.float32
BF16 = mybir.dt.bfloat16

B, D, T = 16, 512, 2048
N_D, K, SUB_D = 4, 512, 128
NCORES = 8
BPC = B // NCORES          # batches per core = 2
ROWS = BPC * T             # rows per core = 4096
N_TOTAL = B * T            # 32768


def _build_nc():
    nc = bacc.Bacc("TRN2", target_bir_lowering=False, debug=False,
                   num_devices=NCORES)
    x = nc.dram_tensor("x", [BPC, D, T], FP32, kind="ExternalInput")
    gum = nc.dram_tensor("gum", [N_D, ROWS, K], FP32, kind="ExternalInput")
    w = nc.dram_tensor("w", [N_D, K, SUB_D], FP32, kind="ExternalInput")
    quant = nc.dram_tensor("quant", [BPC, D, T], FP32, kind="ExternalOutput")
    sums = nc.dram_tensor("sums", [N_D, K], FP32, kind="ExternalOutput")

    xa, ga, wa, qa, sa = x.ap(), gum.ap(), w.ap(), quant.ap(), sums.ap()

    with TileContext(nc) as tc:
        with (
            tc.tile_pool(name="const", bufs=1) as constp,
            tc.tile_pool(name="wg", bufs=2) as wgp,
            tc.tile_pool(name="xin", bufs=2) as xp,
            tc.tile_pool(name="gin", bufs=3) as gp,
            tc.tile_pool(name="work", bufs=3) as wp,
            tc.tile_pool(name="qo", bufs=3) as qp,
            tc.tile_pool(name="small", bufs=8) as sp,
            tc.tile_pool(name="ps", bufs=1, space="PSUM") as psp,
        ):
            ident = constp.tile([128, 128], BF16)
            make_identity(nc, ident)
            ones_f32 = constp.tile([128, 1], FP32)
            nc.vector.memset(ones_f32, 1.0)
            ones_bf = constp.tile([1, 128], BF16)
            nc.vector.memset(ones_bf, 1.0)

            for g in range(N_D):
                # ---- per-group setup: codebook tiles + column norms ----
                w_f32 = wgp.tile([128, 4, 128], FP32, tag="w_f32")
                nc.sync.dma_start(
                    out=w_f32,
                    in_=wa[g].rearrange("(blk p) d -> p blk d", p=128),
                )
                # w_bf[p, blk, d] = W[blk*128+p, d]  (quant lhsT slices)
                w_bf = wgp.tile([128, 4, 128], BF16, tag="w_bf")
                nc.vector.tensor_copy(w_bf, w_f32)
                # wgT[d, blk, p] = W[blk*128+p, d]; view [128, 512] = W^T
                wgT = wgp.tile([128, 4, 128], BF16, tag="wgT")
                for blk in range(4):
                    tps = psp.tile([128, 128], BF16, tag="eT", bufs=2)
                    nc.tensor.transpose(tps, w_bf[:, blk, :], ident)
                    nc.scalar.copy(wgT[:, blk, :], tps)
                wgT_v = wgT.rearrange("p blk d -> p (blk d)")
                # cs_neg_half[1, k] = -||W_k||^2 / 2 via ones^T @ (W^T)^2
                sq = wgp.tile([128, 4, 128], FP32, tag="sq")
                nc.vector.tensor_mul(sq, wgT, wgT)
                cs_ps = psp.tile([1, 512], FP32, tag="qps", bufs=1)
                nc.tensor.matmul(cs_ps, lhsT=ones_f32,
                                 rhs=sq.rearrange("p blk d -> p (blk d)"),
                                 start=True, stop=True)
                csnh = wgp.tile([1, 512], BF16, tag="csnh")
                nc.scalar.mul(csnh, cs_ps, -0.5)

                # per-group avg accumulator (one PSUM bank, 32 matmul accum)
                avg_ps = psp.tile([1, 512], FP32, tag="avg", bufs=1)

                n_tiles = BPC * (T // 128)          # 32 row-tiles per group
                for b in range(BPC):
                    xg_f32 = xp.tile([128, T], FP32, tag="xg_f32")
                    nc.sync.dma_start(out=xg_f32,
                                      in_=xa[b, g * 128:(g + 1) * 128, :])
                    xg_bf = xp.tile([128, T], BF16, tag="xg_bf")
                    nc.vector.tensor_copy(xg_bf, xg_f32)
                    for i in range(T // 128):       # 16 row-tiles per batch
                        it = b * (T // 128) + i     # tile index in group
                        t0 = i * 128
                        n0 = b * T + t0
                        j = it % 2                  # pair slot for stt batch
                        if j == 0:
                            lps2 = psp.tile([128, 2, 512], FP32,
                                            tag="lps", bufs=2)
                            gum2 = gp.tile([128, 2, 512], FP32, tag="gum2")
                        nc.sync.dma_start(out=gum2[:, j, :],
                                          in_=ga[g, n0:n0 + 128, :])
                        nc.tensor.matmul(lps2[:, j, :],
                                         lhsT=xg_bf[:, t0:t0 + 128],
                                         rhs=wgT_v, start=True, stop=False)
                        nc.tensor.matmul(lps2[:, j, :], lhsT=ones_bf,
                                         rhs=csnh, start=False, stop=True)
                        if j == 1:
                            # u = gum/2 + (x.W - cs/2), both tiles at once
                            u2 = wp.tile([128, 2, 512], FP32, tag="u2")
                            nc.vector.scalar_tensor_tensor(
                                out=u2, in0=gum2, scalar=0.5, in1=lps2,
                                op0=mybir.AluOpType.mult,
                                op1=mybir.AluOpType.add)
                            e2 = wp.tile([128, 2, 512], BF16, tag="e2")
                            for jj in range(2):
                                itj = it - 1 + jj
                                s_f32 = sp.tile([128, 1], FP32, tag="s")
                                nc.scalar.activation(
                                    e2[:, jj, :], u2[:, jj, :],
                                    mybir.ActivationFunctionType.Exp,
                                    accum_out=s_f32)
                                r_f32 = sp.tile([128, 1], FP32, tag="r")
                                nc.vector.reciprocal(r_f32, s_f32)
                                r_bf = sp.tile([128, 1], BF16, tag="rbf")
                                nc.gpsimd.tensor_copy(r_bf, r_f32)
                                diag = sp.tile([128, 128], BF16, tag="diag")
                                nc.gpsimd.tensor_scalar_mul(diag, ident, r_f32)
                                # normalized transpose: encT = e^T @ diag(r)
                                eT_ps = psp.tile([128, 4, 128], FP32,
                                                 tag="eT", bufs=2)
                                for blk in range(4):
                                    sl = slice(blk * 128, (blk + 1) * 128)
                                    nc.tensor.matmul(eT_ps[:, blk, :],
                                                     lhsT=e2[:, jj, sl],
                                                     rhs=diag,
                                                     start=True, stop=True)
                                eT_sb = wp.tile([128, 4, 128], BF16, tag="eT_sb")
                                if itj % 2 == 0:
                                    nc.vector.tensor_copy(eT_sb, eT_ps)
                                else:
                                    nc.scalar.copy(eT_sb, eT_ps)
                                # avg_probs partial sums: sum_n enc = r^T @ e
                                nc.tensor.matmul(avg_ps, lhsT=r_bf,
                                                 rhs=e2[:, jj, :],
                                                 start=(itj == 0),
                                                 stop=(itj == n_tiles - 1))
                                # quant^T[d, n] += W[kblk]^T @ encT[kblk]
                                q = itj % 4
                                if q == 0:
                                    qps = psp.tile([128, 512], FP32,
                                                   tag="qps", bufs=1)
                                for blk in range(4):
                                    nc.tensor.matmul(
                                        qps[:, q * 128:(q + 1) * 128],
                                        lhsT=w_bf[:, blk, :],
                                        rhs=eT_sb[:, blk, :],
                                        start=(blk == 0), stop=(blk == 3))
                                if q == 3:
                                    qout = qp.tile([128, 512], FP32, tag="qout")
                                    nc.scalar.copy(qout, qps)
                                    tc0 = (itj // 4) % 4 * 512
                                    bb = itj // (T // 128)
                                    nc.sync.dma_start(
                                        out=qa[bb, g * 128:(g + 1) * 128,
                                               tc0:tc0 + 512],
                                        in_=qout)
                # end of group: drain avg accumulator
                sums_sb = wgp.tile([1, 512], FP32, tag="sums_sb")
                nc.scalar.copy(sums_sb, avg_ps)
                nc.sync.dma_start(out=sa[g:g + 1, :], in_=sums_sb)

    nc.compile()
    return nc


_nc_lock = threading.Lock()
_nc_cache = []


def _get_nc():
    with _nc_lock:
        if not _nc_cache:
            _nc_cache.append(_build_nc())
        return _nc_cache[0]


def kernel(x, W1, W2, W3, W4, gumbels, _trace=False):
    nc = _get_nc()
    W = np.ascontiguousarray(np.stack([W1, W2, W3, W4], 0), dtype=np.float32)
    in_maps = []
    for c in range(NCORES):
        in_maps.append({
            "x": np.ascontiguousarray(x[BPC * c:BPC * (c + 1)]),
            "gum": np.ascontiguousarray(gumbels[:, ROWS * c:ROWS * (c + 1), :]),
            "w": W,
        })
    res = run_bass_kernel_spmd(nc, in_maps, core_ids=list(range(NCORES)),
                               trace=_trace)
    quant = np.empty((B, D, T), np.float32)
    sums_total = np.zeros((N_D, K), np.float64)
    for c, r in enumerate(res.results):
        quant[BPC * c:BPC * (c + 1)] = r["quant"]
        sums_total += r["sums"]
    avg = sums_total / float(N_TOTAL)
    perp = np.sum(np.exp(-np.sum(avg * np.log(avg + 1e-10), axis=-1)))
    out = quant, np.float32(perp)
    if _trace:
        return out, res
    return out


# revision 4
# speedup vs baseline: 1.7848x; 1.6269x over previous
"""Gumbel-softmax VQ codebook kernel for 8 TRN2 NeuronCores.

Math (per group g of 4, sub_D=128, K=512, rows n = flattened B*T):
    dis[g,n,k]  = ||W_g[k]||^2 + ||x_g[n]||^2 - 2 x_g[n].W_g[k]
    enc         = softmax_k((-dis + gum)/tau),  tau = 2
    quant[g,n]  = enc[g,n,:] @ W_g
    avg_probs   = mean_n enc ;  perp = sum_g exp(-sum_k avg log(avg+1e-10))

Design notes:
  * softmax is shift-invariant, so the ||x||^2 term (constant in k) drops;
    logits/tau = x.W - ||W||^2/2 + gum/2 <= ~8.5, so exp() is safe in fp32
    without a max-subtraction pass.
  * x arrives [B, D, T]: x[b, g*128:(g+1)*128, :] IS x_g^T (d on partitions)
    -- no input transposes anywhere.
  * enc^T (for the quant matmul) comes from a PE matmul against diag(1/s),
    fusing softmax normalization into the transpose.
  * avg_probs rides a rank-1 PE matmul (lhsT = 1/s) accumulating into one
    PSUM bank per group; host combines the 8 per-core partial sums.
  * per-instruction overheads dominate small ops, so: the e^T PSUM->SBUF
    copy is ONE op alternating DVE/ACT, quant matmuls/copies batch 4
    row-tiles (512-wide streams), the gum+logits add is pair-batched on
    DVE, and x casts go to the idle GpSimd engine.

Sharding: data-parallel over rows; core c takes batches {2c, 2c+1}
(= flat rows [c*4096, (c+1)*4096)); codebooks replicated.
"""

import threading

import numpy as np

import concourse.bacc as bacc
from concourse import mybir
from concourse.bass_utils import run_bass_kernel_spmd
from concourse.masks import make_identity
from concourse.tile import TileContext

FP32 = mybir.dt.float32
BF16 = mybir.dt.bfloat16

B, D, T = 16, 512, 2048
N_D, K, SUB_D = 4, 512, 128
NCORES = 8
BPC = B // NCORES          # batches per core = 2
ROWS = BPC * T             # rows per core = 4096
N_TOTAL = B * T            # 32768


def _build_nc():
    nc = bacc.Bacc("TRN2", target_bir_lowering=False, debug=False,
                   num_devices=NCORES)
    x = nc.dram_tensor("x", [BPC, D, T], FP32, kind="ExternalInput")
    gum = nc.dram_tensor("gum", [N_D, ROWS, K], FP32, kind="ExternalInput")
    w = nc.dram_tensor("w", [N_D, K, SUB_D], FP32, kind="ExternalInput")
    quant = nc.dram_tensor("quant", [BPC, D, T], FP32, kind="ExternalOutput")
    sums = nc.dram_tensor("sums", [N_D, K], FP32, kind="ExternalOutput")

    xa, ga, wa, qa, sa = x.ap(), gum.ap(), w.ap(), quant.ap(), sums.ap()

    with TileContext(nc) as tc:
        with (
            tc.tile_pool(name="const", bufs=1) as constp,
            tc.tile_pool(name="wg", bufs=2) as wgp,
            tc.tile_pool(name="xin", bufs=2) as xp,
            tc.tile_pool(name="gin", bufs=3) as gp,
            tc.tile_pool(name="work", bufs=3) as wp,
            tc.tile_pool(name="qo", bufs=3) as qp,
            tc.tile_pool(name="small", bufs=8) as sp,
            tc.tile_pool(name="ps", bufs=1, space="PSUM") as psp,
        ):
            ident = constp.tile([128, 128], BF16)
            make_identity(nc, ident)
            ones_f32 = constp.tile([128, 1], FP32)
            nc.vector.memset(ones_f32, 1.0)
            ones_bf = constp.tile([1, 128], BF16)
            nc.vector.memset(ones_bf, 1.0)

            for g in range(N_D):
                # ---- per-group setup: codebook tiles + column norms ----
                w_f32 = wgp.tile([128, 4, 128], FP32, tag="w_f32")
                nc.sync.dma_start(
                    out=w_f32,
                    in_=wa[g].rearrange("(blk p) d -> p blk d", p=128),
                )
                # w_bf[p, blk, d] = W[blk*128+p, d]  (quant lhsT slices)
                w_bf = wgp.tile([128, 4, 128], BF16, tag="w_bf")
                nc.vector.tensor_copy(w_bf, w_f32)
                # wgT[d, blk, p] = W[blk*128+p, d]; view [128, 512] = W^T
                wgT = wgp.tile([128, 4, 128], BF16, tag="wgT")
                for blk in range(4):
                    tps = psp.tile([128, 128], BF16, tag="eT", bufs=2)
                    nc.tensor.transpose(tps, w_bf[:, blk, :], ident)
                    nc.scalar.copy(wgT[:, blk, :], tps)
                wgT_v = wgT.rearrange("p blk d -> p (blk d)")
                # cs_neg_half[1, k] = -||W_k||^2 / 2 via ones^T @ (W^T)^2
                sq = wgp.tile([128, 4, 128], FP32, tag="sq")
                nc.vector.tensor_mul(sq, wgT, wgT)
                cs_ps = psp.tile([1, 512], FP32, tag="qps", bufs=1)
                nc.tensor.matmul(cs_ps, lhsT=ones_f32,
                                 rhs=sq.rearrange("p blk d -> p (blk d)"),
                                 start=True, stop=True)
                csnh = wgp.tile([1, 512], BF16, tag="csnh")
                nc.scalar.mul(csnh, cs_ps, -0.5)

                # per-group avg accumulator (one PSUM bank, 32 matmul accum)
                avg_ps = psp.tile([1, 512], FP32, tag="avg", bufs=1)

                n_tiles = BPC * (T // 128)          # 32 row-tiles per group
                for b in range(BPC):
                    xg_f32 = xp.tile([128, T], FP32, tag="xg_f32")
                    nc.sync.dma_start(out=xg_f32,
                                      in_=xa[b, g * 128:(g + 1) * 128, :])
                    xg_bf = xp.tile([128, T], BF16, tag="xg_bf")
                    nc.vector.tensor_copy(xg_bf, xg_f32)
                    for i in range(T // 128):       # 16 row-tiles per batch
                        it = b * (T // 128) + i     # tile index in group
                        t0 = i * 128
                        n0 = b * T + t0
                        j = it % 2                  # pair slot for stt batch
                        if j == 0:
                            lps2 = psp.tile([128, 2, 512], FP32,
                                            tag="lps", bufs=2)
                            gum2 = gp.tile([128, 2, 512], FP32, tag="gum2")
                            nc.sync.dma_start(
                                out=gum2,
                                in_=ga[g].rearrange(
                                    "(nn p) k -> p nn k", p=128
                                )[:, 2 * (n0 // 256):2 * (n0 // 256) + 2, :])
                        nc.tensor.matmul(lps2[:, j, :],
                                         lhsT=xg_bf[:, t0:t0 + 128],
                                         rhs=wgT_v, start=True, stop=False)
                        nc.tensor.matmul(lps2[:, j, :], lhsT=ones_bf,
                                         rhs=csnh, start=False, stop=True)
                        if j == 1:
                            # u = gum/2 + (x.W - cs/2), both tiles at once
                            u2 = wp.tile([128, 2, 512], FP32, tag="u2")
                            nc.vector.scalar_tensor_tensor(
                                out=u2, in0=gum2, scalar=0.5, in1=lps2,
                                op0=mybir.AluOpType.mult,
                                op1=mybir.AluOpType.add)
                            e2 = wp.tile([128, 2, 512], BF16, tag="e2")
                            for jj in range(2):
                                itj = it - 1 + jj
                                q = itj % 4
                                s_f32 = sp.tile([128, 1], FP32, tag="s")
                                nc.scalar.activation(
                                    e2[:, jj, :], u2[:, jj, :],
                                    mybir.ActivationFunctionType.Exp,
                                    accum_out=s_f32)
                                r_f32 = sp.tile([128, 1], FP32, tag="r")
                                nc.vector.reciprocal(r_f32, s_f32)
                                r_bf = sp.tile([128, 1], BF16, tag="rbf")
                                nc.gpsimd.tensor_copy(r_bf, r_f32)
                                diag = sp.tile([128, 128], BF16, tag="diag")
                                nc.vector.tensor_scalar_mul(diag, ident, r_f32)
                                # normalized transpose: encT = e^T @ diag(r)
                                eT_ps = psp.tile([128, 4, 128], FP32,
                                                 tag="eT", bufs=2)
                                for blk in range(4):
                                    sl = slice(blk * 128, (blk + 1) * 128)
                                    nc.tensor.matmul(eT_ps[:, blk, :],
                                                     lhsT=e2[:, jj, sl],
                                                     rhs=diag,
                                                     start=True, stop=True)
                                # eT for 4 row-tiles, quant-batch layout
                                if q == 0:
                                    eT4 = wp.tile([128, 4, 4, 128], BF16,
                                                  tag="eT4")
                                if itj % 2 == 0:
                                    nc.vector.tensor_copy(eT4[:, :, q, :], eT_ps)
                                else:
                                    nc.scalar.copy(eT4[:, :, q, :], eT_ps)
                                # avg_probs partial sums: sum_n enc = r^T @ e
                                nc.tensor.matmul(avg_ps, lhsT=r_bf,
                                                 rhs=e2[:, jj, :],
                                                 start=(itj == 0),
                                                 stop=(itj == n_tiles - 1))
                                if q == 3:
                                    # quant^T[d, n] over 4 row-tiles at once
                                    qps = psp.tile([128, 512], FP32,
                                                   tag="qps", bufs=1)
                                    for blk in range(4):
                                        nc.tensor.matmul(
                                            qps,
                                            lhsT=w_bf[:, blk, :],
                                            rhs=eT4[:, blk, :, :].rearrange(
                                                "p q n -> p (q n)"),
                                            start=(blk == 0), stop=(blk == 3))
                                    qout = qp.tile([128, 512], FP32, tag="qout")
                                    nc.scalar.copy(qout, qps)
                                    tc0 = (itj // 4) % 4 * 512
                                    bb = itj // (T // 128)
                                    nc.sync.dma_start(
                                        out=qa[bb, g * 128:(g + 1) * 128,
                                               tc0:tc0 + 512],
                                        in_=qout)
                # end of group: drain avg accumulator
                sums_sb = wgp.tile([1, 512], FP32, tag="sums_sb")
                nc.scalar.copy(sums_sb, avg_ps)
                nc.sync.dma_start(out=sa[g:g + 1, :], in_=sums_sb)

    nc.compile()
    return nc


_nc_lock = threading.Lock()
_nc_cache = []


def _get_nc():
    with _nc_lock:
        if not _nc_cache:
            _nc_cache.append(_build_nc())
        return _nc_cache[0]


def kernel(x, W1, W2, W3, W4, gumbels, _trace=False):
    nc = _get_nc()
    W = np.ascontiguousarray(np.stack([W1, W2, W3, W4], 0), dtype=np.float32)
    in_maps = []
    for c in range(NCORES):
        in_maps.append({
            "x": np.ascontiguousarray(x[BPC * c:BPC * (c + 1)]),
            "gum": np.ascontiguousarray(gumbels[:, ROWS * c:ROWS * (c + 1), :]),
            "w": W,
        })
    res = run_bass_kernel_spmd(nc, in_maps, core_ids=list(range(NCORES)),
                               trace=_trace)
    quant = np.empty((B, D, T), np.float32)
    sums_total = np.zeros((N_D, K), np.float64)
    for c, r in enumerate(res.results):
        quant[BPC * c:BPC * (c + 1)] = r["quant"]
        sums_total += r["sums"]
    avg = sums_total / float(N_TOTAL)
    perp = np.sum(np.exp(-np.sum(avg * np.log(avg + 1e-10), axis=-1)))
    out = quant, np.float32(perp)
    if _trace:
        return out, res
    return out
